# revision 31
# baseline (speedup 1.0000x reference)
"""Trainium2 Bass kernel for nn_MemEffAttn (T=1024, B=4, E=1024, H=16, D=64).

Sharding (8 cores): head-parallel attention (2 heads x 4 batches per core),
Megatron-style column-sharded Wq/Wk/Wv, row-sharded Wo.  Each core computes a
full-shape partial of the output projection; the host sums the 8 partials
(row-parallel "gather") and reshapes to (T, B, E).

All HBM traffic is fp16 (inputs converted on the host, partial outputs summed
in fp32 on the host); matmuls run in fp16 at full PE rate.

Per-core dataflow (all on-chip except noted):
  1. qT/kT projections emitted *transposed* ([dims, tokens], dims on
     partitions) so the head_dim contraction of the attention matmuls needs no
     on-device transposes at all; v is emitted in natural layout ([tokens,
     dims]) to serve as the stationary operand of P@V.  kT is stored per-head
     in partition-0 tiles (a 64-offset LDWEIGHTS row group costs ~230ns per
     score matmul).
  2. RoPE applied in transposed layout: the partner-row term x[partner(d)]
     comes from a single [128,128] permutation matmul on the PE (cheap)
     instead of a second full projection GEMM; cos/sin tables precomputed on
     the host (attention scale folded into Wq/bq).
  3. Scores are computed transposed (sT[k, tq] = kT.T @ qT); exp runs on ACT
     straight out of PSUM; the host ships exp(attn_bias) so the bias-add
     becomes a pure-fp16 DVE multiply, which also folds in the key-padding
     mask as a per-partition keep factor: p = (exp(s) * keep) * expb.
     oT = v.T @ p accumulates over k-blocks; a ones-column appended to v
     yields the softmax denominator for free.
  4. Output projection emitted transposed ([e, tokens]) so bo is a
     per-partition ACT bias; DMA'd out as a fp16 [1024, 4096] partial.
"""

import os
import sys

for _p in ("/opt/trn_rl_repo", "/root/.axon_site/_ro/trn_rl_repo"):
    if os.path.isdir(_p) and _p not in sys.path:
        sys.path.insert(0, _p)

import numpy as np
from contextlib import ExitStack

import concourse.bass as bass
import concourse.bacc as bacc
import concourse.tile as tile
from concourse import mybir
from concourse.bass_utils import run_bass_kernel_spmd

F32 = mybir.dt.float32
F16 = mybir.dt.float16
U8 = mybir.dt.uint8

E = 1024
H = 16
D = 64
T = 1024
B = 4
P = 128
NCORES = 8
HPC = H // NCORES  # heads per core = 2
TB = T * B  # 4096 tokens, stored b-major on device
NT = TB // 512  # 8 token tiles of 512
VW = 128  # v block width per head (64 data + 1 ones + 63 zero pad)
SCALE = 1.0 / np.sqrt(np.float32(D))  # 0.125

MMDT = F16


def _build_bass():
    nc = bacc.Bacc("TRN2", target_bir_lowering=False, debug=False)

    # ---- per-core external inputs ----
    queryT = nc.dram_tensor("queryT", [E, TB], F16, kind="ExternalInput")
    expbT = nc.dram_tensor("expbT", [B * HPC, T, T], F16, kind="ExternalInput")
    wqT = nc.dram_tensor("wqT", [E, P], F16, kind="ExternalInput")
    wkT = nc.dram_tensor("wkT", [E, P], F16, kind="ExternalInput")
    wvT = nc.dram_tensor("wvT", [E, P], F16, kind="ExternalInput")
    woT = nc.dram_tensor("woT", [P, E], F16, kind="ExternalInput")
    permM = nc.dram_tensor("permM", [P, P], F16, kind="ExternalInput")
    bq_in = nc.dram_tensor("bq", [P, 1], F32, kind="ExternalInput")
    bv_in = nc.dram_tensor("bv", [1, P], F16, kind="ExternalInput")
    bo_in = nc.dram_tensor("bo", [P, 8], F32, kind="ExternalInput")
    cos_k = nc.dram_tensor("cos_k", [P, T], F16, kind="ExternalInput")
    sin_k = nc.dram_tensor("sin_k", [P, T], F16, kind="ExternalInput")
    outT = nc.dram_tensor("outT", [E, TB], F16, kind="ExternalOutput")

    Exp = mybir.ActivationFunctionType.Exp
    Identity = mybir.ActivationFunctionType.Identity
    Amul = mybir.AluOpType.mult

    with tile.TileContext(nc) as tc, ExitStack() as ctx:
        # ---------------- persistent tiles + constants ----------------
        persist = ctx.enter_context(tc.tile_pool(name="persist", bufs=1))
        # qT/kT/v/oT are rings over 2 batches (slot = b % 2)
        # A matmul with operands at base partition 64 runs ~230ns slower per
        # 512 rows (PE tile-position switch), and walrus requires DVE in/out
        # partition ranges to match.  So q^T keeps both heads stacked
        # [128, t], while k^T is stored as one [128, t] tile per head with
        # the OTHER head's rows zeroed: the score matmul contracts all 128
        # partitions (zero rows kill the cross-head terms exactly) with
        # every operand at base partition 0.
        qT_sb = persist.tile([P, 2 * T], MMDT)
        kTz_h = [
            persist.tile([P, 2 * T], MMDT, name=f"kTz{h}") for h in range(HPC)
        ]
        # v natural layout, 128-wide per-head blocks: cols h*128+0:64 data,
        # h*128+64 ones, rest zero pad (so the PV lhsT is a full 128-col load
        # and o lands on PSUM partitions 0:128 with l on row 64)
        v_sb = persist.tile([P, 16, 2 * VW], MMDT)
        oT_sb = persist.tile([P, 2 * T], MMDT)  # attention out^T
        wo_sb = persist.tile([P, 8, P], MMDT)
        bo_sb = persist.tile([P, 8], F32)
        ident_f32 = persist.tile([P, P], F32)
        ident = persist.tile([P, P], MMDT)

        consts = ctx.enter_context(tc.tile_pool(name="consts", bufs=1))
        wq_sb = consts.tile([P, 8, P], MMDT)
        wk_sb = consts.tile([P, 8, P], MMDT)
        wv_sb = consts.tile([P, 8, P], MMDT)
        perm_sb = consts.tile([P, P], MMDT)
        bq_sb = consts.tile([P, 1], F32)
        bv_sb = consts.tile([P, P], F16)  # bv broadcast along partitions
        ck_sb = consts.tile([P, T], F16)
        sk_sb = consts.tile([P, T], F16)

        qry_pool = ctx.enter_context(tc.tile_pool(name="qry", bufs=2))
        t0_pool = ctx.enter_context(tc.tile_pool(name="t0", bufs=2))
        ptmp_pool = ctx.enter_context(tc.tile_pool(name="ptmp", bufs=2))
        bias_pool = ctx.enter_context(tc.tile_pool(name="sbias", bufs=3))
        p_pool = ctx.enter_context(tc.tile_pool(name="pp", bufs=3))
        rcp_pool = ctx.enter_context(tc.tile_pool(name="rcp", bufs=4))
        rbc_pool = ctx.enter_context(tc.tile_pool(name="rbc", bufs=2))
        outb_pool = ctx.enter_context(tc.tile_pool(name="outb", bufs=2))
        pj_psum = ctx.enter_context(tc.tile_pool(name="pj_psum", bufs=2, space="PSUM"))
        s_psum = ctx.enter_context(tc.tile_pool(name="s_psum", bufs=2, space="PSUM"))
        o_psum = ctx.enter_context(tc.tile_pool(name="o_psum", bufs=1, space="PSUM"))

        qry_tiles = {}

        def emit_qry_dma(nt):
            # query stream rides the gpsimd software queue (latency-tolerant,
            # prefetched 2 batches ahead) so its 1 MB bursts never starve the
            # bias stream on the sync queue and issue cost stays off ACT/DVE
            qry = qry_pool.tile([P, 8, 512], MMDT, tag="qry")
            for kh in range(2):
                nc.gpsimd.dma_start(
                    out=qry[:, kh * 4 : (kh + 1) * 4, :],
                    in_=bass.AP(
                        tensor=queryT,
                        offset=kh * 4 * P * TB + nt * 512,
                        ap=[[TB, P], [P * TB, 4], [1, 512]],
                    ),
                )
            qry_tiles[nt] = qry

        def proj_chunks(nt):
            """Generator of small projection work chunks for token tile nt."""
            sl = slice((nt % 4) * 512, (nt % 4) * 512 + 512)
            tsl = slice((nt * 512) % T, (nt * 512) % T + 512)
            qry = qry_tiles[nt]
            state = {}

            def mm8(ps, w_sb):
                for k in range(8):
                    nc.tensor.matmul(
                        ps[:],
                        lhsT=w_sb[:, k, :],
                        rhs=qry[:, k, :],
                        start=(k == 0),
                        stop=(k == 7),
                    )

            for which, wm_sb, bm in (("q", wq_sb, bq_sb), ("k", wk_sb, None)):

                def c_main(wm_sb=wm_sb, which=which):
                    ps_m = pj_psum.tile([P, 512], F32, tag="pj", name=f"pm{which}")
                    state["m"] = ps_m
                    mm8(ps_m, wm_sb)

                def c_swap(bm=bm, which=which):
                    # stage (proj + bias) to SBUF on ACT (Identity activation
                    # carries the per-partition bias for free, and DVE is the
                    # busier engine), then a single permutation matmul
                    # produces the partner-row term for RoPE
                    ps_m = state["m"]
                    t0 = t0_pool.tile([P, 512], MMDT, tag="t0", name=f"t0{which}")
                    state["t0"] = t0
                    if bm is None:
                        nc.scalar.copy(t0[:], ps_m[:])
                    else:
                        nc.scalar.activation(
                            t0[:], ps_m[:], Identity, bias=bm[:], scale=1.0
                        )
                    ps_s = pj_psum.tile([P, 512], F32, tag="pj", name=f"psw{which}")
                    state["s"] = ps_s
                    nc.tensor.matmul(
                        ps_s[:], lhsT=perm_sb[:], rhs=t0[:], start=True, stop=True
                    )

                def c_rope(which=which):
                    t0, ps_s = state["t0"], state["s"]
                    tmp = ptmp_pool.tile([P, 512], MMDT, tag="tmp", name="tmp")
                    nc.vector.tensor_mul(tmp[:], ps_s[:], sk_sb[:, tsl])
                    if which == "q":
                        nc.vector.tensor_mul(qT_sb[:, sl], t0[:], ck_sb[:, tsl])
                        nc.vector.tensor_add(qT_sb[:, sl], qT_sb[:, sl], tmp[:])
                    else:
                        for h in range(HPC):
                            hp = slice(h * D, (h + 1) * D)
                            dst = kTz_h[h][hp, sl]
                            nc.vector.tensor_mul(dst, t0[hp, :], ck_sb[hp, tsl])
                            nc.vector.tensor_add(dst, dst, tmp[hp, :])

                yield c_main
                yield c_swap
                yield c_rope

            def c_vt():
                # v projected transposed ([dims, tokens]), staged to SBUF for
                # PE transposes
                ps_vt = pj_psum.tile([P, 512], F32, tag="pj", name="psvt")
                for k in range(8):
                    nc.tensor.matmul(
                        ps_vt[:],
                        lhsT=wv_sb[:, k, :],
                        rhs=qry[:, k, :],
                        start=(k == 0),
                        stop=(k == 7),
                    )
                vt_sb = ptmp_pool.tile([P, 512], MMDT, tag="vt", name="vt")
                nc.scalar.copy(vt_sb[:], ps_vt[:])
                state["vt"] = vt_sb

            yield c_vt

            for j in range(4):

                def c_vtr(j=j):
                    ti = (nt % 4) * 4 + j
                    vt_sb = state["vt"]
                    psv = pj_psum.tile([P, P], MMDT, tag="pj", name="psv")
                    nc.tensor.transpose(
                        psv[:], vt_sb[:, j * P : (j + 1) * P], ident[:]
                    )
                    nc.vector.tensor_add(v_sb[:, ti, 0:D], psv[:, 0:D], bv_sb[:, 0:D])
                    nc.vector.tensor_add(
                        v_sb[:, ti, VW : VW + D],
                        psv[:, D : 2 * D],
                        bv_sb[:, D : 2 * D],
                    )

                yield c_vtr

        pending = []  # entries: (tag, fn); tag = ("proj", nt) or ("out", b)

        def pump(n):
            for _ in range(n):
                if pending:
                    pending.pop(0)[1]()

        def pump_proj_upto(nt_max):
            while any(t[0] == "proj" and t[1] <= nt_max for t, _ in pending):
                pending.pop(0)[1]()

        # startup DMA order: qry0/qry1 stream on the scalar queue while the
        # sync queue delivers weights + rope tables, then carries the bias
        # stream; small consts ride the gpsimd software queue.
        emit_qry_dma(0)
        nc.sync.dma_start(
            out=wq_sb[:], in_=wqT.ap().rearrange("(c p) m -> p c m", p=P)
        )
        nc.sync.dma_start(out=perm_sb[:], in_=permM[:])
        emit_qry_dma(1)
        nc.sync.dma_start(
            out=wk_sb[:], in_=wkT.ap().rearrange("(c p) m -> p c m", p=P)
        )
        for t_sb, t_dram in ((ck_sb, cos_k), (sk_sb, sin_k)):
            nc.sync.dma_start(out=t_sb[:], in_=t_dram[:])
        nc.sync.dma_start(
            out=wv_sb[:], in_=wvT.ap().rearrange("(c p) m -> p c m", p=P)
        )
        nc.gpsimd.dma_start(out=bq_sb[:], in_=bq_in[:])
        nc.gpsimd.dma_start(
            out=wo_sb[:], in_=woT.ap().rearrange("p (c m) -> p c m", m=P)
        )
        nc.gpsimd.dma_start(out=bo_sb[:], in_=bo_in[:])
        nc.gpsimd.dma_start(
            out=bv_sb[:], in_=bass.AP(tensor=bv_in, offset=0, ap=[[0, P], [1, P]])
        )
        from concourse.masks import make_identity

        make_identity(nc, ident_f32[:])
        nc.vector.tensor_copy(ident[:], ident_f32[:])
        # ones columns + zero pads of v are invariant: write them once
        nc.vector.memset(v_sb[:, :, D : 2 * VW : VW], 1.0)
        nc.vector.memset(v_sb[:, :, D + 1 : VW], 0.0)
        nc.vector.memset(v_sb[:, :, VW + D + 1 : 2 * VW], 0.0)
        # zero the opposite-head rows of the per-head k tiles once
        nc.vector.memset(kTz_h[0][D:P, :], 0.0)
        nc.vector.memset(kTz_h[1][0:D, :], 0.0)

        # prologue: project batch 0's tokens (nt 0, 1) densely
        pending.extend((("proj", 0), c) for c in proj_chunks(0))
        pending.extend((("proj", 1), c) for c in proj_chunks(1))
        pump(len(pending))

        for b in range(B):
            rb = b % 2
            bsl = slice(rb * T, (rb + 1) * T)
            pump_proj_upto(2 * b + 1)  # this batch's q/k/v must be complete
            if b + 1 < B:
                emit_qry_dma(2 * b + 2)
                emit_qry_dma(2 * b + 3)
                pending.extend(
                    (("proj", 2 * b + 2), c) for c in proj_chunks(2 * b + 2)
                )
                pending.extend(
                    (("proj", 2 * b + 3), c) for c in proj_chunks(2 * b + 3)
                )
            for h in range(HPC):
                bh = b * HPC + h
                hsl = slice(h * D, (h + 1) * D)
                o_ps = o_psum.tile([P, T], F32, tag="ops", name="ops")
                lagged = None
                for kbp in range(4):  # bias DMAs batched: 2 k-blocks, 1 MB
                    bias_t = bias_pool.tile([P, 2, T], F16, tag="bias", name="bias")
                    nc.sync.dma_start(
                        out=bias_t[:],
                        in_=bass.AP(
                            tensor=expbT,
                            offset=bh * T * T + kbp * 2 * P * T,
                            ap=[[T, P], [P * T, 2], [1, T]],
                        ),
                    )
                    for j in range(2):
                        kb = kbp * 2 + j
                        s_ps = s_psum.tile([P, T], F32, tag="sps", name="sps")
                        for half in range(2):
                            nc.tensor.matmul(
                                s_ps[:, half * 512 : (half + 1) * 512],
                                lhsT=kTz_h[h][
                                    :, rb * T + kb * P : rb * T + (kb + 1) * P
                                ],
                                rhs=qT_sb[
                                    :, rb * T + half * 512 : rb * T + (half + 1) * 512
                                ],
                                start=True,
                                stop=True,
                            )
                        pump(1)  # keep the PE queue fed while DVE/ACT drain
                        # exp straight out of PSUM on ACT, then one pure-fp16
                        # DVE multiply applies exp(bias) (which carries the
                        # key-padding keep factor, folded in on the host)
                        p_t = p_pool.tile([P, T], MMDT, tag="pt", name="pt")
                        for hf in range(2):
                            hs = slice(hf * 512, (hf + 1) * 512)
                            nc.scalar.activation(p_t[:, hs], s_ps[:, hs], Exp)
                            nc.vector.tensor_mul(
                                p_t[:, hs], p_t[:, hs], bias_t[:, j, hs]
                            )
                        if lagged is not None:
                            pk, pt_prev = lagged
                            for half in range(2):
                                nc.tensor.matmul(
                                    o_ps[:, half * 512 : (half + 1) * 512],
                                    lhsT=v_sb[:, rb * 8 + pk, h * VW : (h + 1) * VW],
                                    rhs=pt_prev[:, half * 512 : (half + 1) * 512],
                                    start=(pk == 0),
                                    stop=(pk == 7),
                                )
                        lagged = (kb, p_t)
                        # front-load the chunk drain so projection work is
                        # done before the batch boundary forces it inline
                        pump(2 if kb < 4 else 1)
                pk, pt_prev = lagged
                for half in range(2):
                    nc.tensor.matmul(
                        o_ps[:, half * 512 : (half + 1) * 512],
                        lhsT=v_sb[:, rb * 8 + pk, h * VW : (h + 1) * VW],
                        rhs=pt_prev[:, half * 512 : (half + 1) * 512],
                        start=(pk == 0),
                        stop=(pk == 7),
                    )
                # fast unnormalized evict releases the o psum slot (l goes to
                # SBUF partition 0 first: rcp_approx_fast miscomputes on a
                # partition-shifted PSUM input); the reciprocal chain +
                # in-place normalize are deferred as a pumped chunk so their
                # DVE burst doesn't delay the next head's p-multiplies
                l_sb = rcp_pool.tile([1, T], F32, tag="lsb", name="lsb")
                nc.vector.tensor_copy(l_sb[:], o_ps[D : D + 1, :])
                nc.scalar.copy(oT_sb[hsl, bsl], o_ps[0:D, :])

                def c_norm(l_sb=l_sb, hsl=hsl, bsl=bsl):
                    # broadcast + normalize both on gpsimd: chained on one
                    # engine and entirely off the DVE p-multiply path
                    rcp_row = rcp_pool.tile([1, T], F32, tag="lsb", name="rrow")
                    nc.vector.reciprocal_approx_fast(rcp_row[:], l_sb[:])
                    rcp_b = rbc_pool.tile([P, T], F32, tag="rbc", name="rbc")
                    nc.gpsimd.partition_broadcast(rcp_b[:], rcp_row[:])
                    nc.gpsimd.tensor_mul(
                        oT_sb[hsl, bsl], oT_sb[hsl, bsl], rcp_b[hsl, :]
                    )

                pending.append((("norm", bh), c_norm))

            # output projection for batch b: queued as pump chunks so it
            # fills the next batch's PE gaps (inline for the last batch)
            def outproj_chunks(b=b):
                # et-quads sharing one [P, 4, 512] tile -> 1 MB output DMAs
                for half in range(2):
                    for eq in range(2):

                        def c_out(half=half, eq=eq, b=b):
                            ob = outb_pool.tile([P, 4, 512], F16, tag="ob", name="ob")
                            for ei in range(4):
                                et = eq * 4 + ei
                                psf = pj_psum.tile(
                                    [P, 512], F32, tag="pj", name="psf"
                                )
                                nc.tensor.matmul(
                                    psf[:],
                                    lhsT=wo_sb[:, et, :],
                                    rhs=oT_sb[
                                        :,
                                        (b % 2) * T + half * 512 : (b % 2) * T
                                        + (half + 1) * 512,
                                    ],
                                    start=True,
                                    stop=True,
                                )
                                if et % 2 == 0:
                                    nc.scalar.activation(
                                        ob[:, ei, :],
                                        psf[:],
                                        Identity,
                                        bias=bo_sb[:, et : et + 1],
                                        scale=1.0,
                                    )
                                else:
                                    nc.vector.tensor_scalar_add(
                                        ob[:, ei, :], psf[:], bo_sb[:, et : et + 1]
                                    )
                            nc.gpsimd.dma_start(
                                out=bass.AP(
                                    tensor=outT,
                                    offset=eq * 4 * P * TB + b * T + half * 512,
                                    ap=[[TB, P], [P * TB, 4], [1, 512]],
                                ),
                                in_=ob[:],
                            )

                        yield c_out

            if b < B - 1:
                pending.extend((("out", b), c) for c in outproj_chunks())
            else:
                pump(len(pending))
                for c in outproj_chunks():
                    c()

    nc.compile()
    return nc


_NC_CACHE = None


def _get_nc():
    global _NC_CACHE
    if _NC_CACHE is None:
        _NC_CACHE = _build_bass()
    return _NC_CACHE


def _rope_tables():
    """cos/sin tables in [dim(128, 2 heads stacked), t] layout.

    Rows 0:32 of each 64-row head block carry -sin, rows 32:64 carry +sin
    (the rotate_half signs, indexed by output row: the permutation matmul
    supplies x[partner(d)]).  The attention scale is folded into Wq/bq.
    """
    d = np.arange(0, D, 2, dtype=np.float32) / np.float32(D)
    inv_freq = (np.float32(1.0) / np.power(np.float32(10000.0), d)).astype(np.float32)
    t = np.arange(T, dtype=np.float32)
    freqs = t[None, :] * inv_freq[:, None]  # [32, T]
    cos_h = np.cos(np.concatenate([freqs, freqs], axis=0)).astype(np.float32)  # [64,T]
    sin_half = np.sin(freqs).astype(np.float32)
    sin_signed = np.concatenate([-sin_half, sin_half], axis=0)  # [64, T]
    cos = np.vstack([cos_h, cos_h])  # [128, T] (2 heads)
    sin = np.vstack([sin_signed, sin_signed])
    return (
        np.ascontiguousarray(cos.astype(np.float16)),
        np.ascontiguousarray(sin.astype(np.float16)),
    )


# partner-row permutation: within each 64-dim head block, row d maps to
# (d+32) % 64
_SWAP = np.concatenate(
    [np.arange(64).reshape(2, 32)[::-1].ravel() + 64 * hh for hh in range(2)]
)


def _perm_matrix():
    m = np.zeros((P, P), dtype=np.float16)
    m[_SWAP, np.arange(P)] = 1.0
    return m


def _make_in_maps(query, attn_bias, key_padding_mask, Wq, bq, Wk, Wv, bv, Wo, bo):
    query = np.asarray(query, dtype=np.float32)
    attn_bias = np.asarray(attn_bias, dtype=np.float32)
    key_padding_mask = np.asarray(key_padding_mask)
    Wq = np.asarray(Wq, dtype=np.float32)
    Wk = np.asarray(Wk, dtype=np.float32)
    Wv = np.asarray(Wv, dtype=np.float32)
    Wo = np.asarray(Wo, dtype=np.float32)
    bq = np.asarray(bq, dtype=np.float32)
    bv = np.asarray(bv, dtype=np.float32)
    bo = np.asarray(bo, dtype=np.float32)

    # shared across cores
    queryT = np.ascontiguousarray(
        query.transpose(2, 1, 0).reshape(E, TB).astype(np.float16)
    )
    # exp(bias) with the key-padding mask folded in as a keep factor:
    # masked keys get expb = 0, dropping them from numerator + denominator
    keep = 1.0 - key_padding_mask.astype(np.float32)  # [B, T]
    expb = (np.exp(attn_bias) * keep[:, None, :, None]).astype(np.float16)
    cos_k, sin_k = _rope_tables()
    permM = _perm_matrix()
    bo_zero = np.zeros((P, 8), dtype=np.float32)
    bo_col = np.ascontiguousarray(bo.reshape(8, P).T)  # [p, echunk]

    in_maps = []
    for c in range(NCORES):
        rsl = slice(c * P, (c + 1) * P)
        in_maps.append(
            {
                "queryT": queryT,
                "expbT": np.ascontiguousarray(
                    expb[:, c * HPC : (c + 1) * HPC].transpose(0, 1, 3, 2)
                ).reshape(B * HPC, T, T),
                "wqT": np.ascontiguousarray(
                    (Wq[rsl, :].T * np.float32(SCALE)).astype(np.float16)
                ),
                "wkT": np.ascontiguousarray(Wk[rsl, :].T.astype(np.float16)),
                "wvT": np.ascontiguousarray(Wv[rsl, :].T.astype(np.float16)),
                "woT": np.ascontiguousarray(Wo[:, rsl].T.astype(np.float16)),
                "permM": permM,
                "bq": np.ascontiguousarray(bq[rsl].reshape(P, 1) * np.float32(SCALE)),
                "bv": np.ascontiguousarray(bv[rsl].reshape(1, P).astype(np.float16)),
                "bo": bo_col if c == 0 else bo_zero,
                "cos_k": cos_k,
                "sin_k": sin_k,
            }
        )
    return in_maps


def _run(inputs, trace=False, **kwargs):
    nc = _get_nc()
    in_maps = _make_in_maps(**inputs)
    res = run_bass_kernel_spmd(
        nc, in_maps, core_ids=list(range(NCORES)), trace=trace, **kwargs
    )
    acc = np.zeros((E, TB), dtype=np.float32)
    for r in res.results:
        acc += r["outT"].astype(np.float32)
    out = np.ascontiguousarray(acc.reshape(E, B, T).transpose(2, 1, 0))
    return out, res


def kernel(**inputs) -> np.ndarray:
    out, _ = _run(inputs, trace=False)
    return out
